# revision 32
# baseline (speedup 1.0000x reference)
"""DiT attention (B=2, S=2048, DIM=1024, H=16, D=64) on 8 TRN2 NeuronCores.

Sharding: data-parallel over B (2) x tensor-parallel over head groups (4),
so each core owns (one batch, 4 heads). The out-projection is computed as
per-core partials over the 256 e-channels each core owns; the host sums the
4 partials per batch and adds out_b (cheaper than an on-device all-reduce).

Device dataflow (per core, all matmuls bf16 with fp32 PSUM accumulation):
  - host supplies hidden^T [DIM,S], W_qkv^T slice [DIM,768], W_out^T slice
    [256,DIM], and sign-folded RoPE tables [64,S] (identity on cores whose
    head group excludes global head 0, keeping the SPMD program uniform)
  - Q^T,K^T [e,s] and V [s,e] projections; biases fused as K=1 matmuls
  - RoPE on local head 0 via pair-swap SBUF DMA + two muls + add
  - per (query-chunk, head): S^T = K^T_tile.T @ Q^T_chunk -> PSUM,
    exp(0.125*x) on ScalarE -> P^T bf16; O^T_aug = [V|1].T @ P^T accumulated
    over key tiles (row 64 = softmax denominator); normalize via
    reciprocal + partition-broadcast DMA + one DVE multiply
  - out projection back to natural [s, dim] layout, partial y -> DRAM fp32
"""

import numpy as np
import ml_dtypes

import concourse.bacc as bacc
import concourse.bass as bass
import concourse.mybir as mybir
import concourse.tile as tile
from concourse.bass_utils import run_bass_kernel_spmd

B, S, DIM, H, D = 2, 2048, 1024, 16, 64
NCORES = 8
GROUPS = 4     # head groups (tensor parallel)
HPG = 4        # heads per group
E = HPG * D    # 256 e-channels per core per projection
P = 128        # partitions
SC = 512       # free-dim chunk for matmuls
NKT = S // P   # 16 key tiles
NQC = S // SC  # 4 query chunks
NST = S // P   # 16 s tiles
BF = mybir.dt.bfloat16
F32 = mybir.dt.float32

_BF_NP = ml_dtypes.bfloat16


def _build_nc(dump=False):
    nc = bacc.Bacc(None, target_bir_lowering=False)

    hT_d = nc.declare_dram_parameter("hT", [DIM, S], BF, isOutput=False)
    wqkvT_d = nc.declare_dram_parameter("wqkvT", [DIM, 3 * E], BF, isOutput=False)
    qkvb_d = nc.declare_dram_parameter("qkvb", [1, 3 * E], BF, isOutput=False)
    qkvbc_d = nc.declare_dram_parameter("qkvb_col", [2 * E, 1], F32, isOutput=False)
    woutT_d = nc.declare_dram_parameter("woutT", [E, DIM], BF, isOutput=False)
    cos_d = nc.declare_dram_parameter("cos_t", [D, S], BF, isOutput=False)
    sin_d = nc.declare_dram_parameter("sin_t", [D, S], BF, isOutput=False)
    perm_d = nc.declare_dram_parameter("perm", [D, D], BF, isOutput=False)
    y_d = nc.declare_dram_parameter("y", [S, DIM], F32, isOutput=True)
    if dump:
        qt_dump = nc.declare_dram_parameter("qt_dump", [E, S], BF, isOutput=True)
        kt_dump = nc.declare_dram_parameter("kt_dump", [E, S], BF, isOutput=True)
        otu_dump = nc.declare_dram_parameter("otu_dump", [D, HPG * S], BF, isOutput=True)
        ot_dump = nc.declare_dram_parameter("ot_dump", [E, S], BF, isOutput=True)
        v_dump = nc.declare_dram_parameter("v_dump", [P, NKT * HPG * P], BF, isOutput=True)

    hT_t = hT_d.ap().rearrange("(t p) s -> t p s", p=P)        # [8,128,S]
    wqkvT_t = wqkvT_d.ap().rearrange("(t p) e -> t p e", p=P)  # [8,128,768]
    woutT_t = woutT_d.ap().rearrange("(t p) o -> t p o", p=P)  # [2,128,DIM]

    with tile.TileContext(nc) as tc:
        import contextlib
        with contextlib.ExitStack() as ctx:
            consts = ctx.enter_context(tc.tile_pool(name="consts", bufs=1))
            ptpool = ctx.enter_context(tc.tile_pool(name="ptpool", bufs=3))
            work = ctx.enter_context(tc.tile_pool(name="work", bufs=2))
            dram = ctx.enter_context(tc.tile_pool(name="dram", bufs=2, space="DRAM"))
            psum = ctx.enter_context(
                tc.tile_pool(name="psum", bufs=2, space="PSUM"))

            # ---- persistent SBUF tensors ----
            hT_sb = consts.tile([P, DIM // P, S], BF, name="hT_sb")
            wqkvT_sb = consts.tile([P, DIM // P, 3 * E], BF, name="wqkvT_sb")
            qkvb_sb = consts.tile([1, 3 * E], BF, name="qkvb_sb")
            qkvbc_sb = consts.tile([P, 2 * E // P, 1], F32, name="qkvbc_sb")
            woutT_sb = consts.tile([P, E // P, DIM], BF, name="woutT_sb")
            cos_sb = consts.tile([D, S], BF, name="cos_sb")
            sin_sb = consts.tile([D, S], BF, name="sin_sb")
            perm_sb = consts.tile([D, D], BF, name="perm_sb")
            ones_sb = consts.tile([1, SC], BF, name="ones_sb")
            # head slots padded to 128 cols so AV LDWEIGHTS takes the
            # fast-weight-load path (needs exactly 128 weight columns)
            V_sb = consts.tile([P, NKT, HPG * P], BF, name="V_sb")
            QT_sb = consts.tile([P, E // P, S], BF, name="QT_sb")
            KT_sb = consts.tile([P, E // P, S], BF, name="KT_sb")
            OT_sb = consts.tile([P, E // P, S], BF, name="OT_sb")
            # head-major staging so every DVE/matmul operand that touches it
            # sits at base partition 0 (hardware mis-handles offset matmul outs)
            OTu_sb = consts.tile([D, HPG, S], BF, name="OTu_sb")
            q0r = consts.tile([D, S], BF, name="q0r")
            k0r = consts.tile([D, S], BF, name="k0r")
            qtmp = consts.tile([D, S], BF, name="qtmp")
            ktmp = consts.tile([D, S], BF, name="ktmp")

            # ---- loads ----
            for t in range(DIM // P):
                nc.sync.dma_start(out=hT_sb[:, t, :], in_=hT_t[t])
                nc.sync.dma_start(out=wqkvT_sb[:, t, :], in_=wqkvT_t[t])
            nc.sync.dma_start(out=qkvb_sb[:, :], in_=qkvb_d.ap())
            for t in range(2 * E // P):
                nc.sync.dma_start(out=qkvbc_sb[:, t, :],
                                  in_=qkvbc_d.ap()[t * P:(t + 1) * P, :])
            for t in range(E // P):
                nc.sync.dma_start(out=woutT_sb[:, t, :], in_=woutT_t[t])
            nc.sync.dma_start(out=cos_sb[:, :], in_=cos_d.ap())
            nc.sync.dma_start(out=sin_sb[:, :], in_=sin_d.ap())
            nc.sync.dma_start(out=perm_sb[:, :], in_=perm_d.ap())
            nc.vector.memset(ones_sb[:, :], 1.0)
            nc.vector.memset(V_sb[:, :, :], 0.0)
            for h in range(HPG):
                nc.vector.memset(V_sb[:, :, h * P + D], 1.0)

            # ---- V projection: V[s, e] += hT.T @ Wv^T, bias as K=1 matmul ----
            for st in range(NST):
                v_ps = psum.tile([P, E], F32, name="v_ps", tag="mm512")
                for kt in range(DIM // P):
                    nc.tensor.matmul(
                        out=v_ps[:, :],
                        lhsT=hT_sb[:, kt, st * P:(st + 1) * P],
                        rhs=wqkvT_sb[:, kt, 2 * E:3 * E],
                        start=(kt == 0), stop=False)
                nc.tensor.matmul(
                    out=v_ps[:, :],
                    lhsT=ones_sb[0:1, 0:P],
                    rhs=qkvb_sb[0:1, 2 * E:3 * E],
                    start=False, stop=True)
                nc.vector.tensor_copy(
                    out=V_sb[:, st, :].rearrange(
                        "p (h c) -> p h c", h=HPG)[:, :, 0:D],
                    in_=v_ps[:, :].rearrange("p (h c) -> p h c", h=HPG))

            # ---- Q^T / K^T projections: [e, s] ----
            for which, dst in ((0, QT_sb), (1, KT_sb)):
                for et in range(E // P):
                    ecols = slice(which * E + et * P, which * E + (et + 1) * P)
                    for scn in range(NQC):
                        s_sl = slice(scn * SC, (scn + 1) * SC)
                        qk_ps = psum.tile([P, SC], F32, name="qk_ps", tag="mm512")
                        for kt in range(DIM // P):
                            nc.tensor.matmul(
                                out=qk_ps[:, :],
                                lhsT=wqkvT_sb[:, kt, ecols],
                                rhs=hT_sb[:, kt, s_sl],
                                start=(kt == 0), stop=(kt == DIM // P - 1))
                        nc.vector.tensor_scalar_add(
                            out=dst[:, et, s_sl], in0=qk_ps[:, :],
                            scalar1=qkvbc_sb[:, which * 2 + et, :])

            # ---- RoPE on local head 0 (identity tables on non-rope cores) ----
            # pair-swap via TensorE permutation matmul (perm is symmetric),
            # then q0r = q0*cos + swap(q0)*sin_signed on DVE
            nc.vector.tensor_mul(out=qtmp[:, :], in0=QT_sb[0:D, 0, :], in1=cos_sb[:, :])
            nc.vector.tensor_mul(out=ktmp[:, :], in0=KT_sb[0:D, 0, :], in1=cos_sb[:, :])
            for src, dst, tmp in ((QT_sb, q0r, qtmp), (KT_sb, k0r, ktmp)):
                for scn in range(NQC):
                    s_sl = slice(scn * SC, (scn + 1) * SC)
                    sw_ps = psum.tile([D, SC], F32, name="sw_ps", tag="mm512")
                    nc.tensor.matmul(
                        out=sw_ps[:, :], lhsT=perm_sb[:, :],
                        rhs=src[0:D, 0, s_sl], start=True, stop=True)
                    nc.vector.tensor_mul(
                        out=dst[:, s_sl], in0=sw_ps[:, :], in1=sin_sb[:, s_sl])
                    nc.vector.tensor_add(
                        out=dst[:, s_sl], in0=dst[:, s_sl], in1=tmp[:, s_sl])

            if dump:
                for et in range(E // P):
                    nc.sync.dma_start(
                        out=qt_dump.ap()[et * P:(et + 1) * P, :], in_=QT_sb[:, et, :])
                    nc.sync.dma_start(
                        out=kt_dump.ap()[et * P:(et + 1) * P, :], in_=KT_sb[:, et, :])
                nc.sync.dma_start(out=qt_dump.ap()[0:D, :], in_=q0r[:, :])
                nc.sync.dma_start(out=kt_dump.ap()[0:D, :], in_=k0r[:, :])
                nc.sync.dma_start(
                    out=v_dump.ap().rearrange("p (t c) -> p t c", t=NKT),
                    in_=V_sb[:, :, :])

            # ---- attention: qc outer so out-projection can pipeline ----
            for qc in range(NQC):
                q_sl = slice(qc * SC, (qc + 1) * SC)
                for h in range(HPG):
                    if h == 0:
                        qh, kh = q0r[:, :], k0r[:, :]
                    else:
                        po = (h % 2) * D
                        qh = QT_sb[po:po + D, h // 2, :]
                        kh = KT_sb[po:po + D, h // 2, :]
                    PT = ptpool.tile([P, NKT, SC], BF, name="PT", tag="PT")
                    for kt2 in range(NKT // 2):
                        # two S^T matmuls fill a 2-bank PSUM tile; one wide
                        # exp drains it (the 172-cycle ACT overhead amortizes)
                        s_ps = psum.tile([P, 2, SC], F32, name="s_ps",
                                         tag="s_ps", bufs=2)
                        for j in range(2):
                            kt = 2 * kt2 + j
                            nc.tensor.matmul(
                                out=s_ps[:, j, :],
                                lhsT=kh[:, kt * P:(kt + 1) * P],
                                rhs=qh[:, q_sl],
                                start=True, stop=True)
                        nc.scalar.activation(
                            out=PT[:, 2 * kt2:2 * kt2 + 2, :], in_=s_ps[:, :, :],
                            func=mybir.ActivationFunctionType.Exp,
                            scale=0.125)
                    o_ps = psum.tile([P, SC], F32, name="o_ps", tag="o_ps")
                    for kt in range(NKT):
                        nc.tensor.matmul(
                            out=o_ps[:, :],
                            lhsT=V_sb[:, kt, h * P:(h + 1) * P],
                            rhs=PT[:, kt, :],
                            start=(kt == 0), stop=(kt == NKT - 1))
                    # drain PSUM quickly: unnormalized O^T + reciprocal of
                    # the denominator row, then free the accumulation bank
                    po = (h % 2) * D
                    nc.any.tensor_copy(
                        out=OTu_sb[:, h, q_sl], in_=o_ps[0:D, :])
                    # custom-DVE bitwise ops give garbage on a PSUM read path
                    # (HW-only; sim is clean) — stage the row in SBUF first
                    denr = work.tile([1, SC], F32, name="denr", tag="denr", bufs=2)
                    nc.vector.tensor_copy(out=denr[:, :], in_=o_ps[D:D + 1, :])
                    rcp = work.tile([1, SC], F32, name="rcp", tag="rcp", bufs=2)
                    nc.vector.reciprocal_approx_fast(
                        out=rcp[:, :], in_=denr[:, :])
                    if qc < NQC - 1:
                        # partition-broadcast 1/denom via a DRAM bounce (SBUF
                        # APs cannot step-0 over partitions); keeps the PE out
                        # of the normalize chain while it has dense work
                        rcp_dr = dram.tile([1, SC], F32, name="rcp_dr",
                                           tag="rcp_dr", bufs=2)
                        nc.sync.dma_start(out=rcp_dr[:, :], in_=rcp[:, :])
                        rbc = work.tile([D, SC], F32, name="rbc", tag="rbc",
                                        bufs=2)
                        nc.gpsimd.dma_start(
                            out=rbc[:, :],
                            in_=rcp_dr[0:1, :].to_broadcast([D, SC]))
                        nc.vector.tensor_mul(
                            out=OT_sb[po:po + D, h // 2, q_sl],
                            in0=OTu_sb[:, h, q_sl],
                            in1=rbc[:, :])
                    else:
                        # tail: PE is idle here — broadcast via a K=1 matmul
                        # to cut the DMA round-trip off the critical path
                        rcpb = work.tile([1, SC], BF, name="rcpb", tag="rcpb",
                                         bufs=2)
                        nc.vector.tensor_copy(out=rcpb[:, :], in_=rcp[:, :])
                        rbc_ps = psum.tile([D, SC], F32, name="rbc_ps",
                                           tag="mm512")
                        nc.tensor.matmul(
                            out=rbc_ps[:, :],
                            lhsT=ones_sb[0:1, 0:D], rhs=rcpb[:, :],
                            start=True, stop=True)
                        nc.vector.tensor_mul(
                            out=OT_sb[po:po + D, h // 2, q_sl],
                            in0=OTu_sb[:, h, q_sl],
                            in1=rbc_ps[:, :])

                # ---- out projection for the s-tiles of this chunk ----
                for st in range(qc * (SC // P), (qc + 1) * (SC // P)):
                    y_sb = work.tile([P, DIM], F32, name="y_sb", tag="y_sb")
                    for oc in range(DIM // SC):
                        y_ps = psum.tile([P, SC], F32, name="y_ps", tag="mm512")
                        for et in range(E // P):
                            nc.tensor.matmul(
                                out=y_ps[:, :],
                                lhsT=OT_sb[:, et, st * P:(st + 1) * P],
                                rhs=woutT_sb[:, et, oc * SC:(oc + 1) * SC],
                                start=(et == 0), stop=(et == E // P - 1))
                        nc.any.tensor_copy(
                            out=y_sb[:, oc * SC:(oc + 1) * SC], in_=y_ps[:, :])
                    nc.sync.dma_start(
                        out=y_d.ap()[st * P:(st + 1) * P, :], in_=y_sb[:, :])
            if dump:
                nc.sync.dma_start(
                    out=otu_dump.ap().rearrange("d (h s) -> d h s", h=HPG),
                    in_=OTu_sb[:, :, :])
                for et in range(E // P):
                    nc.sync.dma_start(
                        out=ot_dump.ap()[et * P:(et + 1) * P, :], in_=OT_sb[:, et, :])

    if dump:
        nc2 = nc
        # late dumps happen after all writes thanks to Tile deps
    return nc


def _shard_inputs(hidden_states, cos, sin, qkv_w, qkv_b, out_w):
    """Host-side prep: per-core transposed bf16 shards."""
    hs = np.asarray(hidden_states, dtype=np.float32)
    cos = np.asarray(cos, dtype=np.float32)
    sin = np.asarray(sin, dtype=np.float32)
    qkv_w = np.asarray(qkv_w, dtype=np.float32)
    qkv_b = np.asarray(qkv_b, dtype=np.float32)
    out_w = np.asarray(out_w, dtype=np.float32)

    def bf(x):
        return np.ascontiguousarray(x).astype(_BF_NP)

    hT_b = [bf(hs[b].T) for b in range(B)]
    in_maps = []
    for core in range(NCORES):
        b, g = divmod(core, GROUPS)
        e0 = E * g
        wq = qkv_w[e0:e0 + E]
        wk = qkv_w[H * D + e0:H * D + e0 + E]
        wv = qkv_w[2 * H * D + e0:2 * H * D + e0 + E]
        wqkvT = bf(np.concatenate([wq, wk, wv], axis=0).T)      # [DIM, 768]
        qkvb = bf(np.concatenate([
            qkv_b[e0:e0 + E], qkv_b[H * D + e0:H * D + e0 + E],
            qkv_b[2 * H * D + e0:2 * H * D + e0 + E]])[None, :])  # [1, 768]
        qkvb_col = np.ascontiguousarray(np.concatenate([
            qkv_b[e0:e0 + E], qkv_b[H * D + e0:H * D + e0 + E]]
        )[:, None].astype(np.float32))  # [512, 1] q|k bias as column
        woutT = bf(out_w[:, e0:e0 + E].T)                        # [256, DIM]
        if g == 0:
            c = cos[b].T
            sgn = np.where(np.arange(D) % 2 == 0, -1.0, 1.0)[:, None].astype(np.float32)
            s_ = sin[b].T * sgn
        else:
            c = np.ones((D, S), np.float32)
            s_ = np.zeros((D, S), np.float32)
        perm = np.zeros((D, D), np.float32)
        perm[np.arange(D), np.arange(D) ^ 1] = 1.0
        in_maps.append({
            "hT": hT_b[b],
            "wqkvT": wqkvT,
            "qkvb": qkvb,
            "qkvb_col": qkvb_col,
            "woutT": woutT,
            "cos_t": bf(c),
            "sin_t": bf(s_),
            "perm": bf(perm),
        })
    return in_maps


_last_results = None


def _ensure_axon_hooks():
    """run_bass_kernel_spmd imports antenv.axon_hooks when BASS_TRACE is set;
    this image's antenv lacks that module. Provide a no-op stand-in (hook=None
    -> tracing is skipped, run proceeds) so a stray BASS_TRACE can't crash."""
    try:
        import antenv.axon_hooks  # noqa: F401
    except ImportError:
        import sys as _sys
        import types as _types
        try:
            import antenv
        except ImportError:
            return
        mod = _types.ModuleType("antenv.axon_hooks")
        _state = {"hook": None}
        mod.set_axon_ntff_profile_hook = lambda h: _state.__setitem__("hook", h)
        mod.get_axon_ntff_profile_hook = lambda: _state["hook"]
        _sys.modules["antenv.axon_hooks"] = mod
        antenv.axon_hooks = mod


def kernel(hidden_states, cos, sin, qkv_w, qkv_b, out_w, out_b):
    global _last_results
    _ensure_axon_hooks()
    in_maps = _shard_inputs(hidden_states, cos, sin, qkv_w, qkv_b, out_w)
    nc = _build_nc()
    nc.compile()  # Bacc defers register allocation to compile()
    res = run_bass_kernel_spmd(nc, in_maps, core_ids=list(range(NCORES)))
    _last_results = res
    ys = [np.asarray(res.results[c]["y"], dtype=np.float32) for c in range(NCORES)]
    out_b = np.asarray(out_b, dtype=np.float32)
    out = np.stack([
        ys[0] + ys[1] + ys[2] + ys[3] + out_b[None, :],
        ys[4] + ys[5] + ys[6] + ys[7] + out_b[None, :],
    ])
    return out.astype(np.float32)


if __name__ == "__main__":
    nc = _build_nc()
    n_inst = sum(len(bb.instructions) for f in nc.m.functions for bb in f.blocks)
    print(f"built nc with {n_inst} instructions")


# revision 33
# speedup vs baseline: 1.0494x; 1.0494x over previous
"""DiT attention (B=2, S=2048, DIM=1024, H=16, D=64) on 8 TRN2 NeuronCores.

Sharding: data-parallel over B (2) x tensor-parallel over head groups (4),
so each core owns (one batch, 4 heads). The out-projection is computed as
per-core partials over the 256 e-channels each core owns; the host sums the
4 partials per batch and adds out_b (cheaper than an on-device all-reduce).

Device dataflow (per core, all matmuls bf16 with fp32 PSUM accumulation):
  - host supplies hidden^T [DIM,S], W_qkv^T slice [DIM,768], W_out^T slice
    [256,DIM], and sign-folded RoPE tables [64,S] (identity on cores whose
    head group excludes global head 0, keeping the SPMD program uniform)
  - Q^T,K^T [e,s] and V [s,e] projections; biases fused as K=1 matmuls
  - RoPE on local head 0 via pair-swap SBUF DMA + two muls + add
  - per (query-chunk, head): S^T = K^T_tile.T @ Q^T_chunk -> PSUM,
    exp(0.125*x) on ScalarE -> P^T bf16; O^T_aug = [V|1].T @ P^T accumulated
    over key tiles (row 64 = softmax denominator); normalize via
    reciprocal + partition-broadcast DMA + one DVE multiply
  - out projection back to natural [s, dim] layout, partial y -> DRAM fp32
"""

import numpy as np
import ml_dtypes

import concourse.bacc as bacc
import concourse.bass as bass
import concourse.mybir as mybir
import concourse.tile as tile
from concourse.bass_utils import run_bass_kernel_spmd

B, S, DIM, H, D = 2, 2048, 1024, 16, 64
NCORES = 8
GROUPS = 4     # head groups (tensor parallel)
HPG = 4        # heads per group
E = HPG * D    # 256 e-channels per core per projection
P = 128        # partitions
SC = 512       # free-dim chunk for matmuls
NKT = S // P   # 16 key tiles
NQC = S // SC  # 4 query chunks
NST = S // P   # 16 s tiles
BF = mybir.dt.bfloat16
F32 = mybir.dt.float32

_BF_NP = ml_dtypes.bfloat16


def _build_nc(dump=False):
    nc = bacc.Bacc(None, target_bir_lowering=False)

    hT_d = nc.declare_dram_parameter("hT", [DIM, S], BF, isOutput=False)
    wqkvT_d = nc.declare_dram_parameter("wqkvT", [DIM, 3 * E], BF, isOutput=False)
    qkvb_d = nc.declare_dram_parameter("qkvb", [1, 3 * E], BF, isOutput=False)
    qkvbc_d = nc.declare_dram_parameter("qkvb_col", [2 * E, 1], F32, isOutput=False)
    woutT_d = nc.declare_dram_parameter("woutT", [E, DIM], BF, isOutput=False)
    cos_d = nc.declare_dram_parameter("cos_t", [D, S], BF, isOutput=False)
    sin_d = nc.declare_dram_parameter("sin_t", [D, S], BF, isOutput=False)
    perm_d = nc.declare_dram_parameter("perm", [D, D], BF, isOutput=False)
    y_d = nc.declare_dram_parameter("y", [S, DIM], F32, isOutput=True)
    if dump:
        qt_dump = nc.declare_dram_parameter("qt_dump", [E, S], BF, isOutput=True)
        kt_dump = nc.declare_dram_parameter("kt_dump", [E, S], BF, isOutput=True)
        otu_dump = nc.declare_dram_parameter("otu_dump", [D, HPG * S], BF, isOutput=True)
        ot_dump = nc.declare_dram_parameter("ot_dump", [E, S], BF, isOutput=True)
        v_dump = nc.declare_dram_parameter("v_dump", [P, NKT * HPG * P], BF, isOutput=True)

    hT_t = hT_d.ap().rearrange("(t p) s -> t p s", p=P)        # [8,128,S]
    wqkvT_t = wqkvT_d.ap().rearrange("(t p) e -> t p e", p=P)  # [8,128,768]
    woutT_t = woutT_d.ap().rearrange("(t p) o -> t p o", p=P)  # [2,128,DIM]

    with tile.TileContext(nc) as tc:
        import contextlib
        with contextlib.ExitStack() as ctx:
            consts = ctx.enter_context(tc.tile_pool(name="consts", bufs=1))
            ptpool = ctx.enter_context(tc.tile_pool(name="ptpool", bufs=2))
            work = ctx.enter_context(tc.tile_pool(name="work", bufs=2))
            dram = ctx.enter_context(tc.tile_pool(name="dram", bufs=2, space="DRAM"))
            psum = ctx.enter_context(
                tc.tile_pool(name="psum", bufs=2, space="PSUM"))

            # ---- persistent SBUF tensors ----
            hT_sb = consts.tile([P, DIM // P, S], BF, name="hT_sb")
            wqkvT_sb = consts.tile([P, DIM // P, 3 * E], BF, name="wqkvT_sb")
            qkvb_sb = consts.tile([1, 3 * E], BF, name="qkvb_sb")
            qkvbc_sb = consts.tile([P, 2 * E // P, 1], F32, name="qkvbc_sb")
            woutT_sb = consts.tile([P, E // P, DIM], BF, name="woutT_sb")
            cos_sb = consts.tile([D, S], BF, name="cos_sb")
            sin_sb = consts.tile([D, S], BF, name="sin_sb")
            perm_sb = consts.tile([D, D], BF, name="perm_sb")
            ones_sb = consts.tile([1, SC], BF, name="ones_sb")
            # head slots padded to 128 cols so AV LDWEIGHTS takes the
            # fast-weight-load path (needs exactly 128 weight columns)
            V_sb = consts.tile([P, NKT, HPG * P], BF, name="V_sb")
            QT_sb = consts.tile([P, E // P, S], BF, name="QT_sb")
            KT_sb = consts.tile([P, E // P, S], BF, name="KT_sb")
            OT_sb = consts.tile([P, E // P, S], BF, name="OT_sb")
            # head-major staging so every DVE/matmul operand that touches it
            # sits at base partition 0 (hardware mis-handles offset matmul outs)
            OTu_sb = consts.tile([D, HPG, S], BF, name="OTu_sb")
            q0r = consts.tile([D, S], BF, name="q0r")
            k0r = consts.tile([D, S], BF, name="k0r")
            qtmp = consts.tile([D, S], BF, name="qtmp")
            ktmp = consts.tile([D, S], BF, name="ktmp")

            # ---- loads ----
            for t in range(DIM // P):
                nc.sync.dma_start(out=hT_sb[:, t, :], in_=hT_t[t])
                nc.sync.dma_start(out=wqkvT_sb[:, t, :], in_=wqkvT_t[t])
            nc.sync.dma_start(out=qkvb_sb[:, :], in_=qkvb_d.ap())
            for t in range(2 * E // P):
                nc.sync.dma_start(out=qkvbc_sb[:, t, :],
                                  in_=qkvbc_d.ap()[t * P:(t + 1) * P, :])
            for t in range(E // P):
                nc.sync.dma_start(out=woutT_sb[:, t, :], in_=woutT_t[t])
            nc.sync.dma_start(out=cos_sb[:, :], in_=cos_d.ap())
            nc.sync.dma_start(out=sin_sb[:, :], in_=sin_d.ap())
            nc.sync.dma_start(out=perm_sb[:, :], in_=perm_d.ap())
            nc.vector.memset(ones_sb[:, :], 1.0)
            nc.vector.memset(V_sb[:, :, :], 0.0)
            for h in range(HPG):
                nc.vector.memset(V_sb[:, :, h * P + D], 1.0)

            # ---- V projection: V[s, e] += hT.T @ Wv^T, bias as K=1 matmul ----
            for st in range(NST):
                v_ps = psum.tile([P, E], F32, name="v_ps", tag="mm512")
                for kt in range(DIM // P):
                    nc.tensor.matmul(
                        out=v_ps[:, :],
                        lhsT=hT_sb[:, kt, st * P:(st + 1) * P],
                        rhs=wqkvT_sb[:, kt, 2 * E:3 * E],
                        start=(kt == 0), stop=False)
                nc.tensor.matmul(
                    out=v_ps[:, :],
                    lhsT=ones_sb[0:1, 0:P],
                    rhs=qkvb_sb[0:1, 2 * E:3 * E],
                    start=False, stop=True)
                nc.vector.tensor_copy(
                    out=V_sb[:, st, :].rearrange(
                        "p (h c) -> p h c", h=HPG)[:, :, 0:D],
                    in_=v_ps[:, :].rearrange("p (h c) -> p h c", h=HPG))

            # ---- Q^T / K^T projections: [e, s] ----
            for which, dst in ((0, QT_sb), (1, KT_sb)):
                for et in range(E // P):
                    ecols = slice(which * E + et * P, which * E + (et + 1) * P)
                    for scn in range(NQC):
                        s_sl = slice(scn * SC, (scn + 1) * SC)
                        qk_ps = psum.tile([P, SC], F32, name="qk_ps", tag="mm512")
                        for kt in range(DIM // P):
                            nc.tensor.matmul(
                                out=qk_ps[:, :],
                                lhsT=wqkvT_sb[:, kt, ecols],
                                rhs=hT_sb[:, kt, s_sl],
                                start=(kt == 0), stop=(kt == DIM // P - 1))
                        nc.vector.tensor_scalar_add(
                            out=dst[:, et, s_sl], in0=qk_ps[:, :],
                            scalar1=qkvbc_sb[:, which * 2 + et, :])

            # ---- RoPE on local head 0 (identity tables on non-rope cores) ----
            # pair-swap via TensorE permutation matmul (perm is symmetric),
            # then q0r = q0*cos + swap(q0)*sin_signed on DVE
            nc.vector.tensor_mul(out=qtmp[:, :], in0=QT_sb[0:D, 0, :], in1=cos_sb[:, :])
            nc.vector.tensor_mul(out=ktmp[:, :], in0=KT_sb[0:D, 0, :], in1=cos_sb[:, :])
            for src, dst, tmp in ((QT_sb, q0r, qtmp), (KT_sb, k0r, ktmp)):
                for scn in range(NQC):
                    s_sl = slice(scn * SC, (scn + 1) * SC)
                    sw_ps = psum.tile([D, SC], F32, name="sw_ps", tag="mm512")
                    nc.tensor.matmul(
                        out=sw_ps[:, :], lhsT=perm_sb[:, :],
                        rhs=src[0:D, 0, s_sl], start=True, stop=True)
                    nc.vector.tensor_mul(
                        out=dst[:, s_sl], in0=sw_ps[:, :], in1=sin_sb[:, s_sl])
                    nc.vector.tensor_add(
                        out=dst[:, s_sl], in0=dst[:, s_sl], in1=tmp[:, s_sl])

            if dump:
                for et in range(E // P):
                    nc.sync.dma_start(
                        out=qt_dump.ap()[et * P:(et + 1) * P, :], in_=QT_sb[:, et, :])
                    nc.sync.dma_start(
                        out=kt_dump.ap()[et * P:(et + 1) * P, :], in_=KT_sb[:, et, :])
                nc.sync.dma_start(out=qt_dump.ap()[0:D, :], in_=q0r[:, :])
                nc.sync.dma_start(out=kt_dump.ap()[0:D, :], in_=k0r[:, :])
                nc.sync.dma_start(
                    out=v_dump.ap().rearrange("p (t c) -> p t c", t=NKT),
                    in_=V_sb[:, :, :])

            # ---- attention: qc outer so out-projection can pipeline ----
            for qc in range(NQC):
                q_sl = slice(qc * SC, (qc + 1) * SC)
                for h in range(HPG):
                    if h == 0:
                        qh, kh = q0r[:, :], k0r[:, :]
                    else:
                        po = (h % 2) * D
                        qh = QT_sb[po:po + D, h // 2, :]
                        kh = KT_sb[po:po + D, h // 2, :]
                    PT = ptpool.tile([P, NKT, SC], BF, name="PT", tag="PT")
                    for kt2 in range(NKT // 2):
                        # two S^T matmuls fill a 2-bank PSUM tile; one wide
                        # exp drains it (the 172-cycle ACT overhead amortizes)
                        s_ps = psum.tile([P, 2, SC], F32, name="s_ps",
                                         tag="s_ps", bufs=2)
                        for j in range(2):
                            kt = 2 * kt2 + j
                            nc.tensor.matmul(
                                out=s_ps[:, j, :],
                                lhsT=kh[:, kt * P:(kt + 1) * P],
                                rhs=qh[:, q_sl],
                                start=True, stop=True)
                        nc.scalar.activation(
                            out=PT[:, 2 * kt2:2 * kt2 + 2, :], in_=s_ps[:, :, :],
                            func=mybir.ActivationFunctionType.Exp,
                            scale=0.125)
                    o_ps = psum.tile([P, SC], F32, name="o_ps", tag="o_ps")
                    for kt in range(NKT):
                        nc.tensor.matmul(
                            out=o_ps[:, :],
                            lhsT=V_sb[:, kt, h * P:(h + 1) * P],
                            rhs=PT[:, kt, :],
                            start=(kt == 0), stop=(kt == NKT - 1))
                    # drain PSUM quickly: unnormalized O^T + reciprocal of
                    # the denominator row, then free the accumulation bank
                    po = (h % 2) * D
                    nc.any.tensor_copy(
                        out=OTu_sb[:, h, q_sl], in_=o_ps[0:D, :])
                    # custom-DVE bitwise ops give garbage on a PSUM read path
                    # (HW-only; sim is clean) — stage the row in SBUF first
                    denr = work.tile([1, SC], F32, name="denr", tag="denr", bufs=4)
                    nc.vector.tensor_copy(out=denr[:, :], in_=o_ps[D:D + 1, :])
                    rcp = work.tile([1, SC], F32, name="rcp", tag="rcp", bufs=4)
                    nc.vector.reciprocal_approx_fast(
                        out=rcp[:, :], in_=denr[:, :])
                    if qc < NQC - 1:
                        # partition-broadcast 1/denom via a DRAM bounce (SBUF
                        # APs cannot step-0 over partitions); keeps the PE out
                        # of the normalize chain while it has dense work
                        rcp_dr = dram.tile([1, SC], F32, name="rcp_dr",
                                           tag="rcp_dr", bufs=4)
                        nc.sync.dma_start(out=rcp_dr[:, :], in_=rcp[:, :])
                        rbc = work.tile([D, SC], F32, name="rbc", tag="rbc",
                                        bufs=4)
                        nc.gpsimd.dma_start(
                            out=rbc[:, :],
                            in_=rcp_dr[0:1, :].to_broadcast([D, SC]))
                        nc.vector.tensor_mul(
                            out=OT_sb[po:po + D, h // 2, q_sl],
                            in0=OTu_sb[:, h, q_sl],
                            in1=rbc[:, :])
                    else:
                        # tail: PE is idle here — broadcast via a K=1 matmul
                        # to cut the DMA round-trip off the critical path
                        rcpb = work.tile([1, SC], BF, name="rcpb", tag="rcpb",
                                         bufs=4)
                        nc.vector.tensor_copy(out=rcpb[:, :], in_=rcp[:, :])
                        rbc_ps = psum.tile([D, SC], F32, name="rbc_ps",
                                           tag="mm512")
                        nc.tensor.matmul(
                            out=rbc_ps[:, :],
                            lhsT=ones_sb[0:1, 0:D], rhs=rcpb[:, :],
                            start=True, stop=True)
                        nc.vector.tensor_mul(
                            out=OT_sb[po:po + D, h // 2, q_sl],
                            in0=OTu_sb[:, h, q_sl],
                            in1=rbc_ps[:, :])

                # ---- out projection for the s-tiles of this chunk ----
                for st in range(qc * (SC // P), (qc + 1) * (SC // P)):
                    y_sb = work.tile([P, DIM], F32, name="y_sb", tag="y_sb")
                    for oc in range(DIM // SC):
                        y_ps = psum.tile([P, SC], F32, name="y_ps", tag="mm512")
                        for et in range(E // P):
                            nc.tensor.matmul(
                                out=y_ps[:, :],
                                lhsT=OT_sb[:, et, st * P:(st + 1) * P],
                                rhs=woutT_sb[:, et, oc * SC:(oc + 1) * SC],
                                start=(et == 0), stop=(et == E // P - 1))
                        nc.any.tensor_copy(
                            out=y_sb[:, oc * SC:(oc + 1) * SC], in_=y_ps[:, :])
                    nc.sync.dma_start(
                        out=y_d.ap()[st * P:(st + 1) * P, :], in_=y_sb[:, :])
            if dump:
                nc.sync.dma_start(
                    out=otu_dump.ap().rearrange("d (h s) -> d h s", h=HPG),
                    in_=OTu_sb[:, :, :])
                for et in range(E // P):
                    nc.sync.dma_start(
                        out=ot_dump.ap()[et * P:(et + 1) * P, :], in_=OT_sb[:, et, :])

    if dump:
        nc2 = nc
        # late dumps happen after all writes thanks to Tile deps
    return nc


def _shard_inputs(hidden_states, cos, sin, qkv_w, qkv_b, out_w):
    """Host-side prep: per-core transposed bf16 shards."""
    hs = np.asarray(hidden_states, dtype=np.float32)
    cos = np.asarray(cos, dtype=np.float32)
    sin = np.asarray(sin, dtype=np.float32)
    qkv_w = np.asarray(qkv_w, dtype=np.float32)
    qkv_b = np.asarray(qkv_b, dtype=np.float32)
    out_w = np.asarray(out_w, dtype=np.float32)

    def bf(x):
        return np.ascontiguousarray(x).astype(_BF_NP)

    hT_b = [bf(hs[b].T) for b in range(B)]
    in_maps = []
    for core in range(NCORES):
        b, g = divmod(core, GROUPS)
        e0 = E * g
        wq = qkv_w[e0:e0 + E]
        wk = qkv_w[H * D + e0:H * D + e0 + E]
        wv = qkv_w[2 * H * D + e0:2 * H * D + e0 + E]
        wqkvT = bf(np.concatenate([wq, wk, wv], axis=0).T)      # [DIM, 768]
        qkvb = bf(np.concatenate([
            qkv_b[e0:e0 + E], qkv_b[H * D + e0:H * D + e0 + E],
            qkv_b[2 * H * D + e0:2 * H * D + e0 + E]])[None, :])  # [1, 768]
        qkvb_col = np.ascontiguousarray(np.concatenate([
            qkv_b[e0:e0 + E], qkv_b[H * D + e0:H * D + e0 + E]]
        )[:, None].astype(np.float32))  # [512, 1] q|k bias as column
        woutT = bf(out_w[:, e0:e0 + E].T)                        # [256, DIM]
        if g == 0:
            c = cos[b].T
            sgn = np.where(np.arange(D) % 2 == 0, -1.0, 1.0)[:, None].astype(np.float32)
            s_ = sin[b].T * sgn
        else:
            c = np.ones((D, S), np.float32)
            s_ = np.zeros((D, S), np.float32)
        perm = np.zeros((D, D), np.float32)
        perm[np.arange(D), np.arange(D) ^ 1] = 1.0
        in_maps.append({
            "hT": hT_b[b],
            "wqkvT": wqkvT,
            "qkvb": qkvb,
            "qkvb_col": qkvb_col,
            "woutT": woutT,
            "cos_t": bf(c),
            "sin_t": bf(s_),
            "perm": bf(perm),
        })
    return in_maps


_last_results = None


def _ensure_axon_hooks():
    """run_bass_kernel_spmd imports antenv.axon_hooks when BASS_TRACE is set;
    this image's antenv lacks that module. Provide a no-op stand-in (hook=None
    -> tracing is skipped, run proceeds) so a stray BASS_TRACE can't crash."""
    try:
        import antenv.axon_hooks  # noqa: F401
    except ImportError:
        import sys as _sys
        import types as _types
        try:
            import antenv
        except ImportError:
            return
        mod = _types.ModuleType("antenv.axon_hooks")
        _state = {"hook": None}
        mod.set_axon_ntff_profile_hook = lambda h: _state.__setitem__("hook", h)
        mod.get_axon_ntff_profile_hook = lambda: _state["hook"]
        _sys.modules["antenv.axon_hooks"] = mod
        antenv.axon_hooks = mod


def kernel(hidden_states, cos, sin, qkv_w, qkv_b, out_w, out_b):
    global _last_results
    _ensure_axon_hooks()
    in_maps = _shard_inputs(hidden_states, cos, sin, qkv_w, qkv_b, out_w)
    nc = _build_nc()
    nc.compile()  # Bacc defers register allocation to compile()
    res = run_bass_kernel_spmd(nc, in_maps, core_ids=list(range(NCORES)))
    _last_results = res
    ys = [np.asarray(res.results[c]["y"], dtype=np.float32) for c in range(NCORES)]
    out_b = np.asarray(out_b, dtype=np.float32)
    out = np.stack([
        ys[0] + ys[1] + ys[2] + ys[3] + out_b[None, :],
        ys[4] + ys[5] + ys[6] + ys[7] + out_b[None, :],
    ])
    return out.astype(np.float32)


if __name__ == "__main__":
    nc = _build_nc()
    n_inst = sum(len(bb.instructions) for f in nc.m.functions for bb in f.blocks)
    print(f"built nc with {n_inst} instructions")
